# revision 34
# baseline (speedup 1.0000x reference)
"""Trainium2 Bass kernel for LowRankOrthogonalProjection.

    out = target @ (I - W W^T) + source @ (W W^T)
        = target + (source - target) @ W @ W.T        (P = W W^T symmetric)

Design — transposed dataflow, no on-chip transposes, no diff tensor:

  Host stages per-core srcT (fp8e4, [D, rows]) and tgtT (bf16) in DMA
  "group-tile" layout: each 1-2 MB DMA group is a contiguous [128, G*rsb]
  block exactly matching its SBUF tile (large transfers: 374+ GB/s vs
  286 GB/s at 1 MB).  The rank-16 projection attenuates source quantization
  by sqrt(R/D) = 1/16, so fp8 source costs ~0.2% output error (tol 2e-2).

  Per row-sub-batch s (NSB=2 pipelined; tgtT tile stays resident):
    Phase A (per 128-row D-chunk c):
        PE:  yT[16, rows] += (16W_c)^T srcT_c + (-16W_c)^T tgtT_c  (PSUM)
        (yT = 16 W^T (src - tgt)^T with zero vector-engine work)
        ACT: yT -> SBUF bf16
    Phase B (per D-chunk c):
        PE:  corrT_c[128, 512] x nj = (W_c/16) yT                  (PSUM)
        ACT/DVE (2:1): drain PSUM -> bf16 staging (few big ops)
        DVE: one [128, 2*rsb] 2x-mode add  outT = corrT + tgtT
        ACT ring: 2 MB outT stores (parallel to SP-ring inputs)
  Host un-transposes outT -> out (f32).

HBM traffic/core: 8 MB src + 16 MB tgt + 16 MB out = 40 MB (vs 80 MB for
the v1 baseline), PE ~84-91 us busy.  Measured 150-159 us/pass serial
(For_i loop differencing; v1 baseline 272 us same metric).
"""

import contextlib

import numpy as np
import ml_dtypes

B, S, D, R = 4, 4096, 4096, 16
N_CORES = 8
ROWS = B * S                 # 16384
RPC = ROWS // N_CORES        # 2048 rows per core
P = 128
DCH = D // P                 # 32 D-chunks
import os as _os
NSB = int(_os.environ.get("KNSB", "2"))      # row sub-batches per core
RSB = RPC // NSB             # rows per sub-batch
SCL = 16.0                   # scale W into fp8-normal range; undone in wt
SRC_G = int(_os.environ.get("KSRCG", "8"))   # src chunks per DMA group (1 MB)
TGT_G = int(_os.environ.get("KTGTG", "8"))   # tgt chunks per DMA group (2 MB)
OUT_G = int(_os.environ.get("KOUTG", "8"))   # out chunks per DMA group (2 MB)

_NC_CACHE = {}


def build_nc(rpc=RPC, reps=1, loop_n=1, mode="full"):
    import concourse.bass as bass
    import concourse.mybir as mybir
    import concourse.tile as tile

    bf16 = mybir.dt.bfloat16
    fp8 = mybir.dt.float8e4
    f32 = mybir.dt.float32

    nc = bass.Bass("TRN2", target_bir_lowering=False)

    nsb = NSB
    rsb = rpc // nsb
    nj = rsb // 512
    nsg = DCH // SRC_G  # src DMA groups per sub-batch
    ntg = DCH // TGT_G  # tgt/out DMA groups per sub-batch

    # All tensors are staged host-side in group-tile layout: each DMA group
    # is a contiguous [128, G*rsb] block matching the SBUF tile exactly
    # (group row p = chunk-major concat of D-rows g*G*128 + gc*128 + p).
    srcT = nc.dram_tensor("srcT", [nsb * nsg * P, SRC_G * rsb], fp8,
                          kind="ExternalInput")
    tgtT = nc.dram_tensor("tgtT", [nsb * ntg * P, TGT_G * rsb], bf16,
                          kind="ExternalInput")
    wsrc = nc.dram_tensor("wsrc", [P, DCH * R], fp8, kind="ExternalInput")
    wneg = nc.dram_tensor("wneg", [P, DCH * R], bf16, kind="ExternalInput")
    wt = nc.dram_tensor("wt", [R, D], bf16, kind="ExternalInput")
    outT = nc.dram_tensor("outT", [nsb * ntg * P, OUT_G * rsb], bf16,
                          kind="ExternalOutput")

    out_eng = nc.gpsimd if _os.environ.get("KOUTENG", "scalar") == "gpsimd" else nc.scalar

    with tile.TileContext(nc) as tc:
        with (
            tc.tile_pool(name="const", bufs=1) as cpool,
            tc.tile_pool(name="tgall", bufs=2) as tgt_pool,
            tc.tile_pool(name="srcp", bufs=2) as src_pool,
            tc.tile_pool(name="corrp", bufs=2) as corr_pool,
            tc.tile_pool(name="outp", bufs=2) as out_pool,
            tc.tile_pool(name="ps_y", bufs=1, space="PSUM") as ps_y,
            tc.tile_pool(name="ps_c", bufs=3, space="PSUM") as ps_c,
        ):
            wsrc_sb = cpool.tile([P, DCH * R], fp8)
            nc.sync.dma_start(wsrc_sb, wsrc[:, :])
            wneg_sb = cpool.tile([P, DCH * R], bf16)
            nc.sync.dma_start(wneg_sb, wneg[:, :])
            wt_sb = cpool.tile([R, D], bf16)
            # first SWDGE use outside the For_i loop (its one-time ISA init
            # breaks walrus codegen when emitted inside a hardware loop)
            nc.gpsimd.dma_start(wt_sb, wt[:, :])
            yt_sb = cpool.tile([R, rpc], bf16)

            loop_cm = tc.For_i(0, loop_n) if loop_n > 1 else contextlib.nullcontext()
            with loop_cm:
                for rep in range(reps):
                    for s in range(nsb):
                        # ---- phase A: yT = SCL * W^T (src - tgt)^T
                        tg_all = tgt_pool.tile([P, DCH * rsb], bf16, tag="tg")
                        yts = [
                            ps_y.tile([R, 512], f32, tag=f"yt{j}", name=f"yt{j}")
                            for j in range(nj)
                        ]
                        src_sb = None
                        for c in range(DCH):
                            if c % SRC_G == 0:
                                src_sb = src_pool.tile(
                                    [P, SRC_G * rsb], fp8, tag="src"
                                )
                                r0 = (s * nsg + c // SRC_G) * P
                                nc.sync.dma_start(src_sb, srcT[r0 : r0 + P, :])
                            if c % TGT_G == 0:
                                r0 = (s * ntg + c // TGT_G) * P
                                nc.sync.dma_start(
                                    tg_all[:, c * rsb : (c + TGT_G) * rsb],
                                    tgtT[r0 : r0 + P, :],
                                )
                            co = (c % SRC_G) * rsb
                            if mode == "dma":
                                continue
                            for j in range(nj):
                                nc.tensor.matmul(
                                    yts[j],
                                    wsrc_sb[:, c * R : (c + 1) * R],
                                    src_sb[:, co + j * 512 : co + (j + 1) * 512],
                                    start=(c == 0),
                                    stop=False,
                                )
                                nc.tensor.matmul(
                                    yts[j],
                                    wneg_sb[:, c * R : (c + 1) * R],
                                    tg_all[
                                        :,
                                        c * rsb + j * 512 : c * rsb + (j + 1) * 512,
                                    ],
                                    start=False,
                                    stop=(c == DCH - 1),
                                )
                        if mode != "dma":
                            for j in range(nj):
                                nc.scalar.copy(
                                    yt_sb[
                                        :, s * rsb + j * 512 : s * rsb + (j + 1) * 512
                                    ],
                                    yts[j],
                                )

                        if mode in ("dma", "aonly"):
                            # store tgt data back as dummy output: same bytes
                            for g in range(ntg):
                                r0 = (s * ntg + g) * P
                                out_eng.dma_start(
                                    outT[r0 : r0 + P, :],
                                    tg_all[:, g * OUT_G * rsb : (g + 1) * OUT_G * rsb],
                                )
                            continue

                        # ---- phase B: corrT = (W/SCL) yT ; outT = corrT + tgtT
                        # ACT drains every PSUM tile into a 2-chunk-wide bf16
                        # staging tile; DVE then does ONE wide 2x add per
                        # chunk-pair (few big DVE ops -- DRAIN amortized).
                        out_sb = None
                        corr4 = None
                        pend = None  # (dram_row0, tile) of finished out group
                        for c in range(DCH):
                            if c % OUT_G == 0:
                                out_sb = out_pool.tile(
                                    [P, OUT_G * rsb], bf16, tag="out"
                                )
                            if c % 2 == 0:
                                corr4 = corr_pool.tile([P, 2 * rsb], bf16, tag="c4")
                            oo = (c % OUT_G) * rsb
                            cps = ps_c.tile([P, nj * 512], f32, tag="cps")
                            for j in range(nj):
                                nc.tensor.matmul(
                                    cps[:, j * 512 : (j + 1) * 512],
                                    wt_sb[:, c * P : (c + 1) * P],
                                    yt_sb[
                                        :,
                                        s * rsb + j * 512 : s * rsb + (j + 1) * 512,
                                    ],
                                    start=True,
                                    stop=True,
                                )
                            # drain PSUM -> bf16 staging; 2 of 3 on ACT, 1 on DVE
                            ceng = nc.scalar.copy if c % 3 < 2 else nc.vector.tensor_copy
                            ceng(corr4[:, (c % 2) * rsb : (c % 2 + 1) * rsb], cps)
                            if c % 2 == 1:
                                c0 = c - 1
                                o0 = (c0 % OUT_G) * rsb
                                nc.vector.tensor_add(
                                    out_sb[:, o0 : o0 + 2 * rsb],
                                    corr4,
                                    tg_all[:, c0 * rsb : c0 * rsb + 2 * rsb],
                                )
                            if c % OUT_G == OUT_G - 1:
                                # emit previous group's store now (one group late
                                # so the ACT ring never stalls on fresh adds)
                                if pend is not None:
                                    out_eng.dma_start(
                                        outT[pend[0] : pend[0] + P, :], pend[1]
                                    )
                                pend = ((s * ntg + c // OUT_G) * P, out_sb)
                        out_eng.dma_start(outT[pend[0] : pend[0] + P, :], pend[1])

    return nc


def split_waits(nc, limit=1):
    """Walrus encodes at most one semaphore wait per instruction.  Hoist
    extra waits onto standalone EventSemaphore instructions."""
    import concourse.mybir as mybir

    nsplit = 0
    for fn in nc.m.functions:
        for blk in fn.blocks:
            new = []
            for ins in blk.instructions:
                si = ins.sync_info
                waits = list(si.on_wait) if si is not None and si.on_wait else []
                if len(waits) > limit:
                    for k, w in enumerate(waits[:-limit]):
                        es = mybir.InstEventSemaphore(
                            name=f"{ins.name}-hw{k}",
                            engine=ins.engine,
                            sync_info=mybir.SyncInfo(on_wait=[w], on_update=[]),
                        )
                        new.append(es)
                        nsplit += 1
                    ins.sync_info = mybir.SyncInfo(
                        on_wait=waits[-limit:],
                        on_update=list(si.on_update or []),
                    )
                new.append(ins)
            blk.instructions[:] = new
    return nsplit


def _get_nc(rpc=RPC, reps=1, loop_n=1, mode=None):
    import os

    if mode is None:
        mode = os.environ.get("KMODE", "full")
    key = (rpc, reps, loop_n, mode)
    if key not in _NC_CACHE:
        nc = build_nc(rpc, reps, loop_n, mode)
        nc.finalize()
        split_waits(nc)
        _NC_CACHE[key] = nc
    return _NC_CACHE[key]


def make_host_inputs(source, target, weight):
    """Cast/transpose/shard host-side; returns per-core in_maps."""
    bf = ml_dtypes.bfloat16
    fp8 = ml_dtypes.float8_e4m3
    src2 = source.reshape(ROWS, D)
    tgt2 = target.reshape(ROWS, D)
    w = np.asarray(weight, np.float32)
    ws = (SCL * w).reshape(DCH, P, R).transpose(1, 0, 2).reshape(P, DCH * R)
    wsrc = np.clip(ws, -240, 240).astype(fp8)
    wneg = np.ascontiguousarray(-ws).astype(bf)
    wt = np.ascontiguousarray(w.T / SCL).astype(bf)

    def blocks(a2, sl, dt, G):
        """[D, RPC] transposed slice -> group-tile layout
        [NSB * (DCH//G) * 128, G*RSB]: each DMA group is one contiguous
        [128, G*RSB] tile with free dim (chunk-in-group, row)."""
        at = np.ascontiguousarray(a2[sl].T)  # [D, RPC]
        ng = DCH // G
        # [D, NSB, RSB] view: at[:, s*RSB + i]
        a4 = at.reshape(ng, G, P, NSB, RSB)
        # -> [NSB, ng, P, G, RSB]
        a5 = np.ascontiguousarray(a4.transpose(3, 0, 2, 1, 4))
        a5 = a5.reshape(NSB * ng * P, G * RSB)
        if dt is fp8:
            return np.clip(a5, -240, 240).astype(dt)
        return a5.astype(dt)

    in_maps = []
    for c in range(N_CORES):
        sl = slice(c * RPC, (c + 1) * RPC)
        in_maps.append(
            {
                "srcT": blocks(src2, sl, fp8, SRC_G),
                "tgtT": blocks(tgt2, sl, bf, TGT_G),
                "wsrc": wsrc,
                "wneg": wneg,
                "wt": wt,
            }
        )
    return in_maps


# test.py can set this to capture profiling info
LAST_RESULT = None
TRACE = False


def kernel(source, target, weight):
    from concourse.bass_utils import run_bass_kernel_spmd

    global LAST_RESULT
    in_maps = make_host_inputs(
        np.asarray(source), np.asarray(target), np.asarray(weight)
    )
    nc = _get_nc()
    res = run_bass_kernel_spmd(
        nc, in_maps, core_ids=list(range(N_CORES)), trace=TRACE
    )
    LAST_RESULT = res
    ntg = DCH // OUT_G
    out = np.empty((ROWS, D), np.float32)
    for c in range(N_CORES):
        oT = res.results[c]["outT"]  # [NSB*ntg*P, OUT_G*RSB] group-tile layout
        o5 = oT.reshape(NSB, ntg, P, OUT_G, RSB)
        # -> [NSB, ntg, OUT_G, P, RSB] -> [NSB, D, RSB]
        oD = o5.transpose(0, 1, 3, 2, 4).reshape(NSB, D, RSB)
        for s in range(NSB):
            out[c * RPC + s * RSB : c * RPC + (s + 1) * RSB] = (
                oD[s].T.astype(np.float32)
            )
    return out.reshape(B, S, D)
